# revision 1
# baseline (speedup 1.0000x reference)
"""DiceLoss kernel v2 for Trainium2 (8 NeuronCores, data-parallel over batch).

Per pixel, pred-set = {c : fp16(X_c) >= fp16-max}. Device computes, per core:
  - psum_p[i, c]  = partial count_p: sums of IND over partitions/chunks
  - gram_c[i, j]  = sum_p OHY_c[p,i-chunk] * IND_c[p,j-chunk] accumulated over
                    chunks/tiles; its diagonal sums to inter[c]
Host computes count_y = bincount(y), folds partials, and the dice ratio.

Engine split per tile ([128 x 512] pixels, 19 classes):
  - gpsimd: SWDGE DMA f32->fp16 cast loads; slices of the max tree
  - DVE: max-tree remainder, fused is_ge broadcast (IND), most one-hot (OHY)
    tensor_scalar ops (4x mode)
  - ACT: label int8->fp16 convert, a few one-hot classes via Abs/Relu pairs
  - PE: count_p via [128,128]-stationary x ones matmuls; inter via Gram
    matmuls (stationary=OHY chunk, moving=IND chunk), diag read on host
"""

import os
import numpy as np

C = 19
B = 16
HW = 512 * 512
NCORES = 8
BPC = B // NCORES
P = 128
S = 512
TILES_PER_PLANE = HW // (P * S)
NTILES = BPC * TILES_PER_PLANE  # 8
EPS = 1e-5
NCHUNK = S // P  # 4 chunks of 128 pixels per class-slice

# knobs
DVE1 = int(os.environ.get("DICE_DVE1", "4"))   # of 9 L1 pairs on DVE
DVE2 = int(os.environ.get("DICE_DVE2", "2"))   # of 4 L2 pairs on DVE
NACT = int(os.environ.get("DICE_NACT", "9"))   # OHY classes on ACT
GMAX = os.environ.get("DICE_GMAX", "0") == "1"  # gpsimd STT-max (unsupported)
GIND = int(os.environ.get("DICE_GIND", "3"))   # IND classes on gpsimd
NGOH = int(os.environ.get("DICE_NGOH", "4"))   # OHY classes on gpsimd
BX = int(os.environ.get("DICE_BX", "3"))       # X pool bufs
PSDMA = os.environ.get("DICE_PSDMA", "0") == "1"  # psum->dram DMA (illegal)

_CACHE = {}


def _build_nc():
    import concourse.bass as bass
    import concourse.mybir as mybir
    from concourse.tile import TileContext

    f32 = mybir.dt.float32
    fp16 = mybir.dt.float16
    i32 = mybir.dt.int32
    i8 = mybir.dt.int8
    Alu = mybir.AluOpType
    Act = mybir.ActivationFunctionType

    nc = bass.Bass(name="dicev2")
    xp = nc.dram_tensor("y_pred", [BPC, C, HW], f32, kind="ExternalInput")
    yl = nc.dram_tensor("y", [BPC, HW], i32, kind="ExternalInput")
    out_cp = nc.dram_tensor("cp_out", [P, C], f32, kind="ExternalOutput")
    gr_i16 = os.environ.get("DICE_GR16", "0") == "1"
    out_gr = nc.dram_tensor(
        "gram_out", [P, 5 * S], mybir.dt.int16 if gr_i16 else f32,
        kind="ExternalOutput",
    )

    act_cls = set(range(NACT))
    goh_cls = set(range(NACT, NACT + NGOH))
    gind_cls = set(range(C - GIND, C))
    n_dve_ind = C - GIND

    with TileContext(nc) as tc:
        with (
            tc.tile_pool(name="xpool", bufs=BX) as xpool,
            tc.tile_pool(name="indpool", bufs=int(os.environ.get("DICE_BI", "2"))) as indpool,
            tc.tile_pool(name="ohpool", bufs=int(os.environ.get("DICE_BO", "3"))) as ohpool,
            tc.tile_pool(name="smpool", bufs=2) as smpool,
            tc.tile_pool(name="treepool", bufs=2) as treepool,
            tc.tile_pool(name="scpool", bufs=4) as scpool,
            tc.tile_pool(name="fixpool", bufs=1) as fixpool,
            tc.tile_pool(name="pspool", bufs=1, space="PSUM") as pspool,
        ):
            ones = fixpool.tile([P, 1], fp16)
            nc.vector.memset(ones[:], 1.0)
            # per-partition bias constants for ACT one-hot classes
            actbias = {}
            for c in sorted(act_cls):
                bt = fixpool.tile([P, 1], f32, name=f"nbias{c}")
                nc.vector.memset(bt[:], float(-c))
                actbias[c] = bt
            one_f32 = fixpool.tile([P, 1], f32, name="onef32")
            nc.vector.memset(one_f32[:], 1.0)

            # layout must match X tiles: partition p of tile (b, tp) holds
            # pixel row tp*128+p, i.e. ylab8[p, b, tp, :] = y[b, tp, p, :]
            ylab8 = fixpool.tile([P, BPC * HW // P], i8)
            nc.gpsimd.dma_start(
                out=ylab8[:].rearrange(
                    "p (b tp s) -> p b tp s", b=BPC, tp=TILES_PER_PLANE
                ),
                in_=yl[:].rearrange(
                    "b (tp p s) -> p b tp s", tp=TILES_PER_PLANE, p=P
                ),
            )
            ylab = fixpool.tile([P, BPC * HW // P], fp16)
            quarter = BPC * HW // P // 4
            ycvt_dve = int(os.environ.get("DICE_YCVT", "2"))
            for q in range(4):
                sl = slice(q * quarter, (q + 1) * quarter)
                if q >= 4 - ycvt_dve:
                    nc.vector.tensor_copy(out=ylab[:, sl], in_=ylab8[:, sl])
                else:
                    nc.scalar.copy(ylab[:, sl], ylab8[:, sl])
            psum_p = pspool.tile([P, S], f32, name="psump")  # full bank
            grams = [
                pspool.tile([P, S], f32, name=f"gram{g}") for g in range(5)
            ]

            # labels: one casting DMA int32 -> int8, one ACT convert per plane
            # prefetch tile 0's X before the label load so DVE starts early
            X_pre = xpool.tile([P, C * S], fp16, tag="X")
            nc.gpsimd.dma_start(
                out=X_pre[:].rearrange("p (c s) -> p c s", s=S),
                in_=xp[0].rearrange("c (n s) -> n c s", s=S)[0:P],
            )

            def emit_ohy(t, OHY):
                yt = ylab[:, t * S : (t + 1) * S]
                for c in range(C):
                    oc = OHY[:, c * S : (c + 1) * S]
                    if c in act_cls:
                        sc = scpool.tile([P, S], fp16, tag="asc")
                        nc.scalar.activation(
                            out=sc[:], in_=yt, func=Act.Abs,
                            bias=actbias[c][:], scale=1.0,
                        )
                        nc.scalar.activation(
                            out=oc, in_=sc[:], func=Act.Relu,
                            bias=one_f32[:], scale=-1.0,
                        )
                    elif c in goh_cls:
                        nc.gpsimd.tensor_scalar(
                            out=oc, in0=yt, scalar1=float(c), scalar2=None,
                            op0=Alu.is_equal,
                        )
                    else:
                        nc.vector.tensor_scalar(
                            out=oc, in0=yt, scalar1=float(c), scalar2=0.0,
                            op0=Alu.is_equal, op1=Alu.add,
                        )

            xtiles = {0: X_pre}
            ohtiles = {}

            def fetch_ohy(t):
                O = ohpool.tile([P, C * S], fp16, tag="OHY")
                emit_ohy(t, O)
                ohtiles[t] = O

            def fetch(t):
                plane = t // TILES_PER_PLANE
                tp = t % TILES_PER_PLANE
                Xt = xpool.tile([P, C * S], fp16, tag="X")
                nc.gpsimd.dma_start(
                    out=Xt[:].rearrange("p (c s) -> p c s", s=S),
                    in_=xp[plane].rearrange("c (n s) -> n c s", s=S)[
                        tp * P : (tp + 1) * P
                    ],
                )
                xtiles[t] = Xt

            fetch_ohy(0)
            fetch(1)
            for t in range(NTILES):
                if t + 2 < NTILES:
                    fetch(t + 2)
                X = xtiles.pop(t)
                IND = indpool.tile([P, C * S], fp16)

                # ---- max tree -> mb [P, S]; dedicated scratch so the
                # tree never waits on PE reads of older IND buffers
                t1 = treepool.tile([P, 9 * S], fp16, tag="t1")
                mb = smpool.tile([P, S], fp16, tag="mb")

                def gmax(out, in0, in1):
                    # Pool rejects TensorTensor max; STT (a+0) max b is legal
                    nc.gpsimd.scalar_tensor_tensor(
                        out=out, in0=in0, scalar=0.0, in1=in1,
                        op0=Alu.add, op1=Alu.max,
                    )

                d1 = (DVE1 if GMAX else 9) * S
                nc.vector.tensor_tensor(
                    out=t1[:, 0:d1], in0=X[:, 0:d1],
                    in1=X[:, 9 * S : 9 * S + d1], op=Alu.max,
                )
                if d1 < 9 * S:
                    gmax(t1[:, d1 : 9 * S], X[:, d1 : 9 * S],
                         X[:, 9 * S + d1 : 18 * S])
                d2 = (DVE2 if GMAX else 4) * S
                nc.vector.tensor_tensor(
                    out=t1[:, 0:d2], in0=t1[:, 0:d2],
                    in1=t1[:, 4 * S : 4 * S + d2], op=Alu.max,
                )
                if d2 < 4 * S:
                    gmax(t1[:, d2 : 4 * S], t1[:, d2 : 4 * S],
                         t1[:, 4 * S + d2 : 8 * S])
                nc.vector.tensor_tensor(
                    out=t1[:, 0 : 2 * S], in0=t1[:, 0 : 2 * S],
                    in1=t1[:, 2 * S : 4 * S], op=Alu.max,
                )
                nc.vector.tensor_tensor(
                    out=t1[:, 0:S], in0=t1[:, 0:S], in1=t1[:, S : 2 * S],
                    op=Alu.max,
                )
                nc.vector.tensor_tensor(
                    out=t1[:, 0:S], in0=t1[:, 0:S], in1=t1[:, 8 * S : 9 * S],
                    op=Alu.max,
                )
                nc.vector.tensor_tensor(
                    out=mb[:], in0=t1[:, 0:S], in1=X[:, 18 * S : 19 * S],
                    op=Alu.max,
                )

                # ---- IND = (X >= mb): fused broadcast on DVE for leading
                # classes, subtract+is_equal on gpsimd for the tail
                mbap = mb[:]
                mbview = bass.AP(
                    mbap.tensor, mbap.offset,
                    [mbap.ap[0], [0, n_dve_ind], mbap.ap[1]],
                )
                nc.vector.tensor_tensor(
                    out=IND[:, 0 : n_dve_ind * S].rearrange(
                        "p (c s) -> p c s", s=S
                    ),
                    in0=X[:, 0 : n_dve_ind * S].rearrange(
                        "p (c s) -> p c s", s=S
                    ),
                    in1=mbview,
                    op=Alu.is_ge,
                )
                for c in sorted(gind_cls):
                    # f32 scratch: fp16 X-mb underflows near ties
                    Dg = scpool.tile([P, S], f32, tag="dsub")
                    nc.gpsimd.tensor_tensor(
                        out=Dg[:], in0=X[:, c * S : (c + 1) * S], in1=mb[:],
                        op=Alu.subtract,
                    )
                    nc.gpsimd.tensor_scalar(
                        out=IND[:, c * S : (c + 1) * S], in0=Dg[:],
                        scalar1=0.0, scalar2=None, op0=Alu.is_equal,
                    )

                if t + 1 < NTILES:
                    fetch_ohy(t + 1)
                OHY = ohtiles.pop(t)

                # ---- PE: count_p and gram accumulation.
                # start/stop once per PSUM bank: start zeroes the whole bank,
                # so only the first matmul touching a bank may set it.
                first = t == 0
                last = t == NTILES - 1
                for c in range(C):
                    gtile = grams[c // 4]
                    goff = (c % 4) * P
                    gfirst = c % 4 == 0
                    glast = c == C - 1 or c % 4 == 3
                    for a in range(NCHUNK):
                        lo = c * S + a * P
                        ind_ck = IND[:, lo : lo + P]
                        nc.tensor.matmul(
                            psum_p[:, c : c + 1], ind_ck, ones[:],
                            start=(first and c == 0 and a == 0),
                            stop=(last and c == C - 1 and a == NCHUNK - 1),
                            skip_group_check=True,
                        )
                        nc.tensor.matmul(
                            gtile[:, goff : goff + P],
                            OHY[:, lo : lo + P], ind_ck,
                            start=(first and gfirst and a == 0),
                            stop=(last and glast and a == NCHUNK - 1),
                            skip_group_check=True,
                        )

            # ---- psums out
            if PSDMA:
                nc.sync.dma_start(out=out_cp[:], in_=psum_p[:])
                for g in range(5):
                    nc.sync.dma_start(
                        out=out_gr[:, g * S : (g + 1) * S], in_=grams[g][:]
                    )
            else:
                # per-bank copy + DMA so each starts as soon as its last
                # matmul retires; copies split across DVE and ACT
                cp_sb = fixpool.tile([P, C], f32)
                nc.scalar.activation(
                    out=cp_sb[:], in_=psum_p[:, 0:C], func=Act.Copy
                )
                nc.sync.dma_start(out=out_cp[:], in_=cp_sb[:])
                gr_sb = fixpool.tile([P, 5 * S], f32)
                for g in range(5):
                    if os.environ.get("DICE_CPDVE", "0") == "1" and g % 2 == 0:
                        nc.vector.tensor_copy(
                            out=gr_sb[:, g * S : (g + 1) * S], in_=grams[g][:]
                        )
                    else:
                        nc.scalar.activation(
                            out=gr_sb[:, g * S : (g + 1) * S], in_=grams[g][:],
                            func=Act.Copy,
                        )
                    (nc.gpsimd if gr_i16 else nc.sync).dma_start(
                        out=out_gr[:, g * S : (g + 1) * S],
                        in_=gr_sb[:, g * S : (g + 1) * S],
                    )
    return nc


def _split_excess_waits(nc, cap=1):
    """walrus codegen only fits `cap` inline sync-waits on most instruction
    structs; move the excess onto standalone EventSemaphore instructions
    executed just before, on the same engine queue."""
    import concourse.mybir as mybir

    n_split = 0
    for fn in nc.m.functions:
        for blk in fn.blocks:
            out = []
            for inst in blk.instructions:
                si = inst.sync_info
                if si is not None and len(si.on_wait) > cap:
                    waits = list(si.on_wait)
                    keep, excess = waits[-cap:], waits[:-cap]
                    for k, w in enumerate(excess):
                        es = mybir.InstEventSemaphore(
                            name=f"{inst.name}_wsplit{k}", ins=[], outs=[]
                        )
                        es.engine = inst.engine
                        es.sync_info = mybir.SyncInfo(on_wait=[w], on_update=[])
                        out.append(es)
                        n_split += 1
                    inst.sync_info = mybir.SyncInfo(
                        on_wait=keep, on_update=list(si.on_update)
                    )
                out.append(inst)
            blk.instructions[:] = out
    return n_split


def _get_nc():
    if "nc" not in _CACHE:
        nc = _build_nc()
        _split_excess_waits(nc)
        _CACHE["nc"] = nc
    return _CACHE["nc"]


def _run_device(y_pred, y, trace=False):
    from concourse.bass_utils import run_bass_kernel_spmd

    nc = _get_nc()
    xp = np.ascontiguousarray(y_pred.reshape(B, C, HW), dtype=np.float32)
    yi = np.ascontiguousarray(y.reshape(B, HW)).astype(np.int32)
    in_maps = []
    for i in range(NCORES):
        in_maps.append(
            {
                "y_pred": np.ascontiguousarray(xp[i * BPC : (i + 1) * BPC]),
                "y": np.ascontiguousarray(yi[i * BPC : (i + 1) * BPC]),
            }
        )
    return run_bass_kernel_spmd(
        nc, in_maps, core_ids=list(range(NCORES)), trace=trace
    )


def kernel(y_pred, y):
    res = _run_device(y_pred, y)
    count_p = np.zeros(C, dtype=np.float64)
    inter = np.zeros(C, dtype=np.float64)
    for r in res.results:
        count_p += r["cp_out"].astype(np.float64).sum(axis=0)
        gr = r["gram_out"].astype(np.float64)
        for c in range(C):
            g = gr[:, (c // 4) * S + (c % 4) * P : (c // 4) * S + (c % 4) * P + P]
            inter[c] += np.trace(g)
    count_y = np.bincount(
        np.asarray(y).reshape(-1).astype(np.int64), minlength=C
    ).astype(np.float64)

    count_p = count_p.astype(np.float32)
    count_y = count_y.astype(np.float32)
    inter = inter.astype(np.float32)
    union = count_y + count_p - inter
    eps = np.float32(EPS)
    dice = (np.float32(2.0) * inter + eps) / (union + eps)
    return np.float32(1.0) - np.mean(dice, dtype=np.float32)



# revision 2
# speedup vs baseline: 1.0266x; 1.0266x over previous
"""DiceLoss kernel v3 for Trainium2 (8 NeuronCores, data-parallel over batch).

Per pixel, pred-set = {c : fp16(X_c) >= fp16-max}. Device computes, per core:
  - psum_p[i, c]  = partial count_p: sums of IND over partitions/chunks
  - gram_c[i, j]  = sum_p OHY_c[p,i-chunk] * IND_c[p,j-chunk] accumulated over
                    chunks/tiles; its diagonal sums to G0[c] * inter[c]
Host computes count_y = bincount(y), folds partials, and the dice ratio.

v3 engine split per tile ([128 x 512] pixels, 19 classes):
  - gpsimd: SWDGE DMA f32->fp16 cast loads; OHY for 4 classes (tensor_scalar
    is_equal, int8 labels in); IND for 2 classes (subtract f32 + is_equal)
  - DVE: whole max tree; fused is_ge broadcast IND for 17 classes
  - ACT: OHY for 15 classes via ONE Derivative_Erf activation each:
    D_Erf(4*(y-c)) = (2/sqrt(pi))*exp(-16*(y-c)^2) -> G0 at y==c, ~1e-7 off.
    Host divides gram traces by G0 (fp16 constant, measured on HW).
  - PE: count_p via [128,128]-stationary x ones matmuls; inter via Gram
    matmuls (stationary=OHY chunk, moving=IND chunk), diag read on host
Labels stay int8 on device (ACT/Pool consume int8 directly) - no fp16
label conversion pass.  Gram psums leave as int16 (entries <= ~4700).
"""

import os
import numpy as np

C = 19
B = 16
HW = 512 * 512
NCORES = 8
BPC = B // NCORES
P = 128
S = 512
TILES_PER_PLANE = HW // (P * S)
NTILES = BPC * TILES_PER_PLANE  # 8
EPS = 1e-5
NCHUNK = S // P  # 4 chunks of 128 pixels per class-slice

# fp16 value of Derivative_Erf(0) = 2/sqrt(pi) as produced by the ACT table
G0_ACT = 1.1279296875

# knobs
NACT = int(os.environ.get("DICE_NACT", "15"))   # OHY classes on ACT (D_Erf)
NGOH = int(os.environ.get("DICE_NGOH", "4"))    # OHY classes on gpsimd
GIND = int(os.environ.get("DICE_GIND", "2"))    # IND classes on gpsimd
BX = int(os.environ.get("DICE_BX", "3"))        # X pool bufs
NSPLIT = int(os.environ.get("DICE_NSPLIT", "2"))  # early tiles w/ split X DMA

_CACHE = {}


def _build_nc():
    import concourse.bass as bass
    import concourse.mybir as mybir
    from concourse.tile import TileContext

    f32 = mybir.dt.float32
    fp16 = mybir.dt.float16
    i16 = mybir.dt.int16
    i8 = mybir.dt.int8
    Alu = mybir.AluOpType
    Act = mybir.ActivationFunctionType

    assert NACT + NGOH == C
    nc = bass.Bass(name="dicev3")
    xp = nc.dram_tensor("y_pred", [BPC, C, HW], f32, kind="ExternalInput")
    yl = nc.dram_tensor("y", [BPC, HW], i32 := mybir.dt.int32, kind="ExternalInput")
    out_cp = nc.dram_tensor("cp_out", [P, C], f32, kind="ExternalOutput")
    out_gr = nc.dram_tensor("gram_out", [P, 5 * S], i16, kind="ExternalOutput")

    act_cls = set(range(NACT))
    goh_cls = set(range(NACT, NACT + NGOH))
    gind_cls = set(range(C - GIND, C))
    n_dve_ind = C - GIND

    with TileContext(nc) as tc:
        with (
            tc.tile_pool(name="xpool", bufs=BX) as xpool,
            tc.tile_pool(name="indpool", bufs=int(os.environ.get("DICE_BI", "2"))) as indpool,
            tc.tile_pool(name="ohpool", bufs=int(os.environ.get("DICE_BO", "3"))) as ohpool,
            tc.tile_pool(name="smpool", bufs=2) as smpool,
            tc.tile_pool(name="treepool", bufs=2) as treepool,
            tc.tile_pool(name="scpool", bufs=4) as scpool,
            tc.tile_pool(name="fixpool", bufs=1) as fixpool,
            tc.tile_pool(name="pspool", bufs=1, space="PSUM") as pspool,
        ):
            ones = fixpool.tile([P, 1], fp16)
            nc.vector.memset(ones[:], 1.0)
            # per-partition bias constants for ACT D_Erf one-hot: -4c
            actbias = {}
            for c in sorted(act_cls):
                bt = fixpool.tile([P, 1], f32, name=f"nbias{c}")
                nc.vector.memset(bt[:], float(-4 * c))
                actbias[c] = bt

            # layout must match X tiles: partition p of tile (b, tp) holds
            # pixel row tp*128+p, i.e. ylab8[p, b, tp, :] = y[b, tp, p, :]
            ylab8 = fixpool.tile([P, BPC * HW // P], i8)
            nc.gpsimd.dma_start(
                out=ylab8[:].rearrange(
                    "p (b tp s) -> p b tp s", b=BPC, tp=TILES_PER_PLANE
                ),
                in_=yl[:].rearrange(
                    "b (tp p s) -> p b tp s", tp=TILES_PER_PLANE, p=P
                ),
            )
            psum_p = pspool.tile([P, S], f32, name="psump")  # full bank
            grams = [
                pspool.tile([P, S], f32, name=f"gram{g}") for g in range(5)
            ]

            def emit_ohy(t, OHY):
                yt = ylab8[:, t * S : (t + 1) * S]
                for c in range(C):
                    oc = OHY[:, c * S : (c + 1) * S]
                    if c in act_cls:
                        nc.scalar.activation(
                            out=oc, in_=yt, func=Act.Derivative_Erf,
                            bias=actbias[c][:], scale=4.0,
                        )
                    else:
                        nc.gpsimd.tensor_scalar(
                            out=oc, in0=yt, scalar1=float(c), scalar2=None,
                            op0=Alu.is_equal,
                        )

            xtiles = {}
            ohtiles = {}

            def fetch_ohy(t):
                O = ohpool.tile([P, C * S], fp16, tag="OHY")
                emit_ohy(t, O)
                ohtiles[t] = O

            def fetch(t, split=False):
                plane = t // TILES_PER_PLANE
                tp = t % TILES_PER_PLANE
                Xt = xpool.tile([P, C * S], fp16, tag="X")
                src = xp[plane].rearrange("c (n s) -> n c s", s=S)[
                    tp * P : (tp + 1) * P
                ]
                dst = Xt[:].rearrange("p (c s) -> p c s", s=S)
                if split:
                    # halves so the tree's A-part can start sooner
                    nc.gpsimd.dma_start(out=dst[:, 0:10], in_=src[:, 0:10])
                    nc.gpsimd.dma_start(out=dst[:, 10:19], in_=src[:, 10:19])
                else:
                    nc.gpsimd.dma_start(out=dst, in_=src)
                xtiles[t] = Xt

            # prefetch X0 (split) before the label DMA would queue-block it;
            # label DMA then X1.. follow
            fetch(0, split=True)
            fetch_ohy(0)
            fetch(1, split=NSPLIT > 1)
            for t in range(NTILES):
                if t + 2 < NTILES:
                    fetch(t + 2, split=t + 2 < NSPLIT)
                X = xtiles.pop(t)
                IND = indpool.tile([P, C * S], fp16)

                # ---- max tree -> mb [P, S]; half-friendly order:
                # A half = classes 0..9, B half = classes 10..18
                t1 = treepool.tile([P, 9 * S], fp16, tag="t1")
                mb = smpool.tile([P, S], fp16, tag="mb")

                # L1A: max(X[0:5], X[5:10]) -> t1[0:5]
                nc.vector.tensor_tensor(
                    out=t1[:, 0 : 5 * S], in0=X[:, 0 : 5 * S],
                    in1=X[:, 5 * S : 10 * S], op=Alu.max,
                )
                # L2A: max(t1[0:2], t1[2:4]) -> t1[0:2]
                nc.vector.tensor_tensor(
                    out=t1[:, 0 : 2 * S], in0=t1[:, 0 : 2 * S],
                    in1=t1[:, 2 * S : 4 * S], op=Alu.max,
                )
                # L3A: max(t1[0], t1[1]) -> t1[0]
                nc.vector.tensor_tensor(
                    out=t1[:, 0:S], in0=t1[:, 0:S], in1=t1[:, S : 2 * S],
                    op=Alu.max,
                )
                # L1B: max(X[10:14], X[14:18]) -> t1[5:9]
                nc.vector.tensor_tensor(
                    out=t1[:, 5 * S : 9 * S], in0=X[:, 10 * S : 14 * S],
                    in1=X[:, 14 * S : 18 * S], op=Alu.max,
                )
                # L2B: max(t1[5:7], t1[7:9]) -> t1[5:7]
                nc.vector.tensor_tensor(
                    out=t1[:, 5 * S : 7 * S], in0=t1[:, 5 * S : 7 * S],
                    in1=t1[:, 7 * S : 9 * S], op=Alu.max,
                )
                # L3B: max(t1[5], t1[6]) -> t1[5]
                nc.vector.tensor_tensor(
                    out=t1[:, 5 * S : 6 * S], in0=t1[:, 5 * S : 6 * S],
                    in1=t1[:, 6 * S : 7 * S], op=Alu.max,
                )
                # L4: max(A, B) -> t1[0]
                nc.vector.tensor_tensor(
                    out=t1[:, 0:S], in0=t1[:, 0:S], in1=t1[:, 5 * S : 6 * S],
                    op=Alu.max,
                )
                # L5: fold carry t1[4] (=max(X4,X9)), then X[18]
                nc.vector.tensor_tensor(
                    out=t1[:, 0:S], in0=t1[:, 0:S], in1=t1[:, 4 * S : 5 * S],
                    op=Alu.max,
                )
                nc.vector.tensor_tensor(
                    out=mb[:], in0=t1[:, 0:S], in1=X[:, 18 * S : 19 * S],
                    op=Alu.max,
                )

                # ---- IND = (X >= mb): fused broadcast on DVE for leading
                # classes, subtract+is_equal on gpsimd for the tail
                mbap = mb[:]
                mbview = bass.AP(
                    mbap.tensor, mbap.offset,
                    [mbap.ap[0], [0, n_dve_ind], mbap.ap[1]],
                )
                nc.vector.tensor_tensor(
                    out=IND[:, 0 : n_dve_ind * S].rearrange(
                        "p (c s) -> p c s", s=S
                    ),
                    in0=X[:, 0 : n_dve_ind * S].rearrange(
                        "p (c s) -> p c s", s=S
                    ),
                    in1=mbview,
                    op=Alu.is_ge,
                )
                for c in sorted(gind_cls):
                    # f32 scratch: fp16 X-mb underflows near ties
                    Dg = scpool.tile([P, S], f32, tag="dsub")
                    nc.gpsimd.tensor_tensor(
                        out=Dg[:], in0=X[:, c * S : (c + 1) * S], in1=mb[:],
                        op=Alu.subtract,
                    )
                    nc.gpsimd.tensor_scalar(
                        out=IND[:, c * S : (c + 1) * S], in0=Dg[:],
                        scalar1=0.0, scalar2=None, op0=Alu.is_equal,
                    )

                if t + 1 < NTILES:
                    fetch_ohy(t + 1)
                OHY = ohtiles.pop(t)

                # ---- PE: count_p and gram accumulation.
                # start/stop once per PSUM bank: start zeroes the whole bank,
                # so only the first matmul touching a bank may set it.
                first = t == 0
                last = t == NTILES - 1
                for c in range(C):
                    gtile = grams[c // 4]
                    goff = (c % 4) * P
                    gfirst = c % 4 == 0
                    glast = c == C - 1 or c % 4 == 3
                    for a in range(NCHUNK):
                        lo = c * S + a * P
                        ind_ck = IND[:, lo : lo + P]
                        nc.tensor.matmul(
                            psum_p[:, c : c + 1], ind_ck, ones[:],
                            start=(first and c == 0 and a == 0),
                            stop=(last and c == C - 1 and a == NCHUNK - 1),
                            skip_group_check=True,
                        )
                        nc.tensor.matmul(
                            gtile[:, goff : goff + P],
                            OHY[:, lo : lo + P], ind_ck,
                            start=(first and gfirst and a == 0),
                            stop=(last and glast and a == NCHUNK - 1),
                            skip_group_check=True,
                        )

            # ---- psums out: per-bank copy + DMA so each starts as soon as
            # its last matmul retires; copies split across DVE and ACT
            cp_sb = fixpool.tile([P, C], f32)
            nc.scalar.activation(
                out=cp_sb[:], in_=psum_p[:, 0:C], func=Act.Copy
            )
            nc.sync.dma_start(out=out_cp[:], in_=cp_sb[:])
            gr_sb = fixpool.tile([P, 5 * S], i16)
            for g in range(5):
                if g % 2 == 0:
                    nc.vector.tensor_copy(
                        out=gr_sb[:, g * S : (g + 1) * S], in_=grams[g][:]
                    )
                else:
                    nc.scalar.activation(
                        out=gr_sb[:, g * S : (g + 1) * S], in_=grams[g][:],
                        func=Act.Copy,
                    )
                nc.sync.dma_start(
                    out=out_gr[:, g * S : (g + 1) * S],
                    in_=gr_sb[:, g * S : (g + 1) * S],
                )
    return nc


def _split_excess_waits(nc, cap=1):
    """walrus codegen only fits `cap` inline sync-waits on most instruction
    structs; move the excess onto standalone EventSemaphore instructions
    executed just before, on the same engine queue."""
    import concourse.mybir as mybir

    n_split = 0
    for fn in nc.m.functions:
        for blk in fn.blocks:
            out = []
            for inst in blk.instructions:
                si = inst.sync_info
                if si is not None and len(si.on_wait) > cap:
                    waits = list(si.on_wait)
                    keep, excess = waits[-cap:], waits[:-cap]
                    for k, w in enumerate(excess):
                        es = mybir.InstEventSemaphore(
                            name=f"{inst.name}_wsplit{k}", ins=[], outs=[]
                        )
                        es.engine = inst.engine
                        es.sync_info = mybir.SyncInfo(on_wait=[w], on_update=[])
                        out.append(es)
                        n_split += 1
                    inst.sync_info = mybir.SyncInfo(
                        on_wait=keep, on_update=list(si.on_update)
                    )
                out.append(inst)
            blk.instructions[:] = out
    return n_split


def _get_nc():
    if "nc" not in _CACHE:
        nc = _build_nc()
        _split_excess_waits(nc)
        _CACHE["nc"] = nc
    return _CACHE["nc"]


def _run_device(y_pred, y, trace=False):
    from concourse.bass_utils import run_bass_kernel_spmd

    nc = _get_nc()
    xp = np.ascontiguousarray(y_pred.reshape(B, C, HW), dtype=np.float32)
    yi = np.ascontiguousarray(y.reshape(B, HW)).astype(np.int32)
    in_maps = []
    for i in range(NCORES):
        in_maps.append(
            {
                "y_pred": np.ascontiguousarray(xp[i * BPC : (i + 1) * BPC]),
                "y": np.ascontiguousarray(yi[i * BPC : (i + 1) * BPC]),
            }
        )
    return run_bass_kernel_spmd(
        nc, in_maps, core_ids=list(range(NCORES)), trace=trace
    )


def kernel(y_pred, y):
    res = _run_device(y_pred, y)
    count_p = np.zeros(C, dtype=np.float64)
    inter = np.zeros(C, dtype=np.float64)
    g0 = np.array([G0_ACT if c < NACT else 1.0 for c in range(C)])
    for r in res.results:
        count_p += r["cp_out"].astype(np.float64).sum(axis=0)
        gr = r["gram_out"].astype(np.float64)
        for c in range(C):
            g = gr[:, (c // 4) * S + (c % 4) * P : (c // 4) * S + (c % 4) * P + P]
            inter[c] += np.trace(g) / g0[c]
    count_y = np.bincount(
        np.asarray(y).reshape(-1).astype(np.int64), minlength=C
    ).astype(np.float64)

    count_p = count_p.astype(np.float32)
    count_y = count_y.astype(np.float32)
    inter = inter.astype(np.float32)
    union = count_y + count_p - inter
    eps = np.float32(EPS)
    dice = (np.float32(2.0) * inter + eps) / (union + eps)
    return np.float32(1.0) - np.mean(dice, dtype=np.float32)


# revision 4
# speedup vs baseline: 1.0537x; 1.0264x over previous
"""DiceLoss kernel v3.1 for Trainium2 (8 NeuronCores, data-parallel over batch).

Per pixel, pred-set = {c : fp16(X_c) >= fp16-max}. Device computes, per core:
  - psum_p[i, c]  = partial count_p: sums of IND over partitions/chunks
  - gram_c[i, j]  = sum_p OHY_c[p,i-chunk] * IND_c[p,j-chunk] accumulated over
                    chunks/tiles; its diagonal sums to G0[c] * inter[c]
Host computes count_y = bincount(y), folds partials, and the dice ratio.

Engine split per tile ([128 x 512] pixels, 19 classes):
  - gpsimd: SWDGE DMA f32->fp16 cast X loads; OHY for NGOH classes
    (tensor_scalar is_equal, int8 labels); subtract half of the sign-route
    IND classes
  - DVE: whole max tree (8/8/3 halves so it can start on partial X tiles);
    fused is_ge broadcast IND for the leading classes
  - ACT: OHY for NACT classes via ONE Derivative_Erf activation each:
    D_Erf(4*(y-c)) = (2/sqrt(pi))*exp(-16*(y-c)^2) -> G0 at y==c, ~1e-7 off.
    Sign(X_c - mb) for the SGN tail IND classes (values {-1,0}; host
    un-biases counts).  Host divides gram traces by G0.
  - PE: count_p via [128,128]-stationary x ones matmuls; inter via Gram
    matmuls (stationary=OHY chunk, moving=IND chunk), diag read on host
  - SP/HWDGE: label DMA (host pre-casts y to int8) + result DMAs
Labels stay int8 on device; gram psums leave as int16 (entries <= ~4700).
"""

import os
import numpy as np

C = 19
B = 16
HW = 512 * 512
NCORES = 8
BPC = B // NCORES
P = 128
S = 512
TILES_PER_PLANE = HW // (P * S)
NTILES = BPC * TILES_PER_PLANE  # 8
EPS = 1e-5
NCHUNK = S // P  # 4 chunks of 128 pixels per class-slice

# fp16 value of Derivative_Erf(0) = 2/sqrt(pi) as produced by the ACT table
G0_ACT = 1.1279296875

# knobs
NACT = int(os.environ.get("DICE_NACT", "14"))   # OHY classes on ACT (D_Erf)
NGOH = C - NACT                                  # OHY classes on gpsimd
SGN = int(os.environ.get("DICE_SGN", "2"))      # sign-route IND classes
BX = int(os.environ.get("DICE_BX", "3"))        # X pool bufs

_CACHE = {}


def _build_nc():
    import concourse.bass as bass
    import concourse.mybir as mybir
    from concourse.tile import TileContext

    f32 = mybir.dt.float32
    fp16 = mybir.dt.float16
    i16 = mybir.dt.int16
    i8 = mybir.dt.int8
    Alu = mybir.AluOpType
    Act = mybir.ActivationFunctionType

    nc = bass.Bass(name="dicev31")
    xp = nc.dram_tensor("y_pred", [BPC, C, HW], f32, kind="ExternalInput")
    yl = nc.dram_tensor("y", [BPC, HW], i8, kind="ExternalInput")
    out_cp = nc.dram_tensor("cp_out", [P, C], f32, kind="ExternalOutput")
    out_gr = nc.dram_tensor("gram_out", [P, 5 * S], i16, kind="ExternalOutput")

    act_cls = set(range(NACT))
    sgn_cls = set(range(C - SGN, C))
    n_dve_ind = C - SGN

    with TileContext(nc) as tc:
        with (
            tc.tile_pool(name="xpool", bufs=BX) as xpool,
            tc.tile_pool(name="indpool", bufs=int(os.environ.get("DICE_BI", "2"))) as indpool,
            tc.tile_pool(name="ohpool", bufs=int(os.environ.get("DICE_BO", "3"))) as ohpool,
            tc.tile_pool(name="smpool", bufs=2) as smpool,
            tc.tile_pool(name="treepool", bufs=2) as treepool,
            tc.tile_pool(name="scpool", bufs=4) as scpool,
            tc.tile_pool(name="fixpool", bufs=1) as fixpool,
            tc.tile_pool(name="pspool", bufs=1, space="PSUM") as pspool,
        ):
            xtiles = {}
            ohtiles = {}

            def fetch(t, parts=((0, 19),)):
                plane = t // TILES_PER_PLANE
                tp = t % TILES_PER_PLANE
                Xt = xpool.tile([P, C * S], fp16, tag="X")
                src = xp[plane].rearrange("c (n s) -> n c s", s=S)[
                    tp * P : (tp + 1) * P
                ]
                dst = Xt[:].rearrange("p (c s) -> p c s", s=S)
                for lo, hi in parts:
                    nc.gpsimd.dma_start(out=dst[:, lo:hi], in_=src[:, lo:hi])
                xtiles[t] = Xt

            # X0 split fine so the tree starts on partial data; label DMA on
            # HWDGE (no cast needed: host sends int8) overlaps X0's SWDGE gen
            fetch(0, parts=((0, 4), (4, 8), (8, 16), (16, 19)))
            ylab8 = fixpool.tile([P, BPC * HW // P], i8)
            nc.sync.dma_start(
                out=ylab8[:].rearrange(
                    "p (b tp s) -> p b tp s", b=BPC, tp=TILES_PER_PLANE
                ),
                in_=yl[:].rearrange(
                    "b (tp p s) -> p b tp s", tp=TILES_PER_PLANE, p=P
                ),
            )
            fetch(1, parts=((0, 8), (8, 19)))

            ones = fixpool.tile([P, 1], fp16)
            nc.vector.memset(ones[:], 1.0)
            # per-partition bias constants for ACT D_Erf one-hot: -4c
            actbias = {}
            for c in sorted(act_cls):
                bt = fixpool.tile([P, 1], f32, name=f"nbias{c}")
                nc.vector.memset(bt[:], float(-4 * c))
                actbias[c] = bt

            psum_p = pspool.tile([P, S], f32, name="psump")  # full bank
            grams = [
                pspool.tile([P, S], f32, name=f"gram{g}") for g in range(5)
            ]

            def emit_ohy(t, OHY):
                yt = ylab8[:, t * S : (t + 1) * S]
                for c in range(C):
                    oc = OHY[:, c * S : (c + 1) * S]
                    if c in act_cls:
                        nc.scalar.activation(
                            out=oc, in_=yt, func=Act.Derivative_Erf,
                            bias=actbias[c][:], scale=4.0,
                        )
                    else:
                        nc.gpsimd.tensor_scalar(
                            out=oc, in0=yt, scalar1=float(c), scalar2=None,
                            op0=Alu.is_equal,
                        )

            def fetch_ohy(t):
                O = ohpool.tile([P, C * S], fp16, tag="OHY")
                emit_ohy(t, O)
                ohtiles[t] = O

            fetch_ohy(0)
            for t in range(NTILES):
                if t + 2 < NTILES:
                    fetch(t + 2)
                X = xtiles.pop(t)
                IND = indpool.tile([P, C * S], fp16)

                # ---- max tree -> mb [P, S]; 8/8/3 split so the A part can
                # run on X[0:8S] alone (matches the split first-tile DMAs)
                t1 = treepool.tile([P, 9 * S], fp16, tag="t1")
                mb = smpool.tile([P, S], fp16, tag="mb")

                def vmax(out, a, b):
                    nc.vector.tensor_tensor(out=out, in0=a, in1=b, op=Alu.max)

                if t == 0:
                    # pair-sequential A half: usable after X[0:4S] even
                    vmax(t1[:, 0:2 * S], X[:, 0:2 * S], X[:, 2 * S:4 * S])
                    vmax(t1[:, 2 * S:4 * S], X[:, 4 * S:6 * S],
                         X[:, 6 * S:8 * S])
                else:
                    vmax(t1[:, 0:4 * S], X[:, 0:4 * S], X[:, 4 * S:8 * S])
                # L2A/L3A -> t1[0]
                vmax(t1[:, 0:2 * S], t1[:, 0:2 * S], t1[:, 2 * S:4 * S])
                vmax(t1[:, 0:S], t1[:, 0:S], t1[:, S:2 * S])
                # B half: classes 8..15 -> t1[4]
                vmax(t1[:, 4 * S:8 * S], X[:, 8 * S:12 * S],
                     X[:, 12 * S:16 * S])
                vmax(t1[:, 4 * S:6 * S], t1[:, 4 * S:6 * S],
                     t1[:, 6 * S:8 * S])
                vmax(t1[:, 4 * S:5 * S], t1[:, 4 * S:5 * S],
                     t1[:, 5 * S:6 * S])
                # C tail: classes 16..18
                vmax(t1[:, 8 * S:9 * S], X[:, 16 * S:17 * S],
                     X[:, 17 * S:18 * S])
                vmax(t1[:, 0:S], t1[:, 0:S], t1[:, 4 * S:5 * S])
                vmax(t1[:, 8 * S:9 * S], t1[:, 8 * S:9 * S],
                     X[:, 18 * S:19 * S])
                vmax(mb[:], t1[:, 0:S], t1[:, 8 * S:9 * S])

                # ---- IND = (X >= mb): fused broadcast on DVE for leading
                # classes; sign-route for the tail: Pool subtract (f32) +
                # ACT Sign -> {-1, 0}; host un-biases the counts
                mbap = mb[:]
                mbview = bass.AP(
                    mbap.tensor, mbap.offset,
                    [mbap.ap[0], [0, n_dve_ind], mbap.ap[1]],
                )
                nc.vector.tensor_tensor(
                    out=IND[:, 0 : n_dve_ind * S].rearrange(
                        "p (c s) -> p c s", s=S
                    ),
                    in0=X[:, 0 : n_dve_ind * S].rearrange(
                        "p (c s) -> p c s", s=S
                    ),
                    in1=mbview,
                    op=Alu.is_ge,
                )
                for c in sorted(sgn_cls):
                    Dg = scpool.tile([P, S], f32, tag="dsub")
                    nc.gpsimd.tensor_tensor(
                        out=Dg[:], in0=X[:, c * S : (c + 1) * S], in1=mb[:],
                        op=Alu.subtract,
                    )
                    nc.scalar.activation(
                        out=IND[:, c * S : (c + 1) * S], in_=Dg[:],
                        func=Act.Sign,
                    )

                if t + 1 < NTILES:
                    fetch_ohy(t + 1)
                OHY = ohtiles.pop(t)

                # ---- PE: count_p and gram accumulation.
                # start/stop once per PSUM bank: start zeroes the whole bank,
                # so only the first matmul touching a bank may set it.
                first = t == 0
                last = t == NTILES - 1
                for c in range(C):
                    gtile = grams[c // 4]
                    goff = (c % 4) * P
                    gfirst = c % 4 == 0
                    glast = c == C - 1 or c % 4 == 3
                    for a in range(NCHUNK):
                        lo = c * S + a * P
                        ind_ck = IND[:, lo : lo + P]
                        nc.tensor.matmul(
                            psum_p[:, c : c + 1], ind_ck, ones[:],
                            start=(first and c == 0 and a == 0),
                            stop=(last and c == C - 1 and a == NCHUNK - 1),
                            skip_group_check=True,
                        )
                        nc.tensor.matmul(
                            gtile[:, goff : goff + P],
                            OHY[:, lo : lo + P], ind_ck,
                            start=(first and gfirst and a == 0),
                            stop=(last and glast and a == NCHUNK - 1),
                            skip_group_check=True,
                        )

            # ---- psums out: per-bank copy + DMA so each starts as soon as
            # its last matmul retires; copies split across DVE and ACT
            cp_sb = fixpool.tile([P, C], f32)
            nc.scalar.activation(
                out=cp_sb[:], in_=psum_p[:, 0:C], func=Act.Copy
            )
            nc.sync.dma_start(out=out_cp[:], in_=cp_sb[:])
            gr_sb = fixpool.tile([P, 5 * S], i16)
            for g in range(5):
                if g % 2 == 0:
                    nc.vector.tensor_copy(
                        out=gr_sb[:, g * S : (g + 1) * S], in_=grams[g][:]
                    )
                else:
                    nc.scalar.activation(
                        out=gr_sb[:, g * S : (g + 1) * S], in_=grams[g][:],
                        func=Act.Copy,
                    )
                nc.sync.dma_start(
                    out=out_gr[:, g * S : (g + 1) * S],
                    in_=gr_sb[:, g * S : (g + 1) * S],
                )
    return nc


def _split_excess_waits(nc, cap=1):
    """walrus codegen only fits `cap` inline sync-waits on most instruction
    structs; move the excess onto standalone EventSemaphore instructions
    executed just before, on the same engine queue."""
    import concourse.mybir as mybir

    n_split = 0
    for fn in nc.m.functions:
        for blk in fn.blocks:
            out = []
            for inst in blk.instructions:
                si = inst.sync_info
                if si is not None and len(si.on_wait) > cap:
                    waits = list(si.on_wait)
                    keep, excess = waits[-cap:], waits[:-cap]
                    for k, w in enumerate(excess):
                        es = mybir.InstEventSemaphore(
                            name=f"{inst.name}_wsplit{k}", ins=[], outs=[]
                        )
                        es.engine = inst.engine
                        es.sync_info = mybir.SyncInfo(on_wait=[w], on_update=[])
                        out.append(es)
                        n_split += 1
                    inst.sync_info = mybir.SyncInfo(
                        on_wait=keep, on_update=list(si.on_update)
                    )
                out.append(inst)
            blk.instructions[:] = out
    return n_split


def _get_nc():
    if "nc" not in _CACHE:
        nc = _build_nc()
        _split_excess_waits(nc)
        _CACHE["nc"] = nc
    return _CACHE["nc"]


def _run_device(y_pred, y, trace=False):
    from concourse.bass_utils import run_bass_kernel_spmd

    nc = _get_nc()
    xp = np.ascontiguousarray(y_pred.reshape(B, C, HW), dtype=np.float32)
    yi = np.ascontiguousarray(y.reshape(B, HW)).astype(np.int8)
    in_maps = []
    for i in range(NCORES):
        in_maps.append(
            {
                "y_pred": np.ascontiguousarray(xp[i * BPC : (i + 1) * BPC]),
                "y": np.ascontiguousarray(yi[i * BPC : (i + 1) * BPC]),
            }
        )
    return run_bass_kernel_spmd(
        nc, in_maps, core_ids=list(range(NCORES)), trace=trace
    )


def kernel(y_pred, y):
    res = _run_device(y_pred, y)
    count_p = np.zeros(C, dtype=np.float64)
    inter = np.zeros(C, dtype=np.float64)
    g0 = np.array([G0_ACT if c < NACT else 1.0 for c in range(C)])
    yi = np.asarray(y).reshape(B, -1).astype(np.int64)
    npix_core = BPC * HW
    for ci, r in enumerate(res.results):
        cp = r["cp_out"].astype(np.float64).sum(axis=0)
        gr = r["gram_out"].astype(np.float64)
        county_core = np.bincount(
            yi[ci * BPC : (ci + 1) * BPC].reshape(-1), minlength=C
        ).astype(np.float64)
        for c in range(C):
            g = gr[:, (c // 4) * S + (c % 4) * P : (c // 4) * S + (c % 4) * P + P]
            tr = np.trace(g) / g0[c]
            if c >= C - SGN:
                # sign-route: IND stored as {-1,0} = IND-1
                cp[c] += npix_core
                tr += county_core[c]
            inter[c] += tr
        count_p += cp
    count_y = np.bincount(yi.reshape(-1), minlength=C).astype(np.float64)

    count_p = count_p.astype(np.float32)
    count_y = count_y.astype(np.float32)
    inter = inter.astype(np.float32)
    union = count_y + count_p - inter
    eps = np.float32(EPS)
    dice = (np.float32(2.0) * inter + eps) / (union + eps)
    return np.float32(1.0) - np.mean(dice, dtype=np.float32)


# revision 9
# speedup vs baseline: 1.1093x; 1.0528x over previous
"""DiceLoss kernel v3.1 for Trainium2 (8 NeuronCores, data-parallel over batch).

Per pixel, pred-set = {c : fp16(X_c) >= fp16-max}. Device computes, per core:
  - psum_p[i, c]  = partial count_p: sums of IND over partitions/chunks
  - gram_c[i, j]  = sum_p OHY_c[p,i-chunk] * IND_c[p,j-chunk] accumulated over
                    chunks/tiles; its diagonal sums to G0[c] * inter[c]
Host computes count_y = bincount(y), folds partials, and the dice ratio.

Engine split per tile ([128 x 512] pixels, 19 classes):
  - gpsimd: SWDGE DMA f32->fp16 cast X loads; OHY for NGOH classes
    (tensor_scalar is_equal, int8 labels); subtract half of the sign-route
    IND classes
  - DVE: whole max tree (8/8/3 halves so it can start on partial X tiles);
    fused is_ge broadcast IND for the leading classes
  - ACT: OHY for NACT classes via ONE Derivative_Erf activation each:
    D_Erf(4*(y-c)) = (2/sqrt(pi))*exp(-16*(y-c)^2) -> G0 at y==c, ~1e-7 off.
    Sign(X_c - mb) for the SGN tail IND classes (values {-1,0}; host
    un-biases counts).  Host divides gram traces by G0.
  - PE: count_p via [128,128]-stationary x ones matmuls; inter via Gram
    matmuls (stationary=OHY chunk, moving=IND chunk), diag read on host
  - SP/HWDGE: label DMA (host pre-casts y to int8) + result DMAs
Labels stay int8 on device; gram psums leave as int16 (entries <= ~4700).
"""

import os
import numpy as np

C = 19
B = 16
HW = 512 * 512
NCORES = 8
BPC = B // NCORES
P = 128
S = 512
TILES_PER_PLANE = HW // (P * S)
NTILES = BPC * TILES_PER_PLANE  # 8
EPS = 1e-5
NCHUNK = S // P  # 4 chunks of 128 pixels per class-slice

# fp16 value of Derivative_Erf(0) = 2/sqrt(pi) as produced by the ACT table
G0_ACT = 1.1279296875

# knobs
NACT = int(os.environ.get("DICE_NACT", "14"))   # OHY classes on ACT (D_Erf)
NGOH = C - NACT                                  # OHY classes on gpsimd
SGN = int(os.environ.get("DICE_SGN", "2"))      # sign-route IND classes
BX = int(os.environ.get("DICE_BX", "3"))        # X pool bufs

_CACHE = {}


def _build_nc():
    import concourse.bass as bass
    import concourse.mybir as mybir
    from concourse.tile import TileContext

    f32 = mybir.dt.float32
    fp16 = mybir.dt.float16
    i16 = mybir.dt.int16
    i8 = mybir.dt.int8
    Alu = mybir.AluOpType
    Act = mybir.ActivationFunctionType

    nc = bass.Bass(name="dicev31")
    xp = nc.dram_tensor("y_pred", [BPC, C, HW], f32, kind="ExternalInput")
    yl = nc.dram_tensor("y", [BPC, HW], i8, kind="ExternalInput")
    out_cp = nc.dram_tensor("cp_out", [P, C], f32, kind="ExternalOutput")
    out_gr = nc.dram_tensor("gram_out", [P, 5 * S], i16, kind="ExternalOutput")

    act_cls = set(range(C - NACT, C))
    sgn_cls = set(range(SGN))

    with TileContext(nc) as tc:
        with (
            tc.tile_pool(name="xpool", bufs=BX) as xpool,
            tc.tile_pool(name="indpool", bufs=int(os.environ.get("DICE_BI", "2"))) as indpool,
            tc.tile_pool(name="ohpool", bufs=int(os.environ.get("DICE_BO", "3"))) as ohpool,
            tc.tile_pool(name="smpool", bufs=2) as smpool,
            tc.tile_pool(name="treepool", bufs=2) as treepool,
            tc.tile_pool(name="scpool", bufs=4) as scpool,
            tc.tile_pool(name="fixpool", bufs=1) as fixpool,
            tc.tile_pool(name="pspool", bufs=1, space="PSUM") as pspool,
        ):
            xtiles = {}
            ohtiles = {}

            def fetch(t, parts=((0, 19),)):
                plane = t // TILES_PER_PLANE
                tp = t % TILES_PER_PLANE
                Xt = xpool.tile([P, C * S], fp16, tag="X")
                src = xp[plane].rearrange("c (n s) -> n c s", s=S)[
                    tp * P : (tp + 1) * P
                ]
                dst = Xt[:].rearrange("p (c s) -> p c s", s=S)
                for lo, hi in parts:
                    nc.gpsimd.dma_start(out=dst[:, lo:hi], in_=src[:, lo:hi])
                xtiles[t] = Xt

            # X0 split fine so the tree starts on partial data; label DMA on
            # HWDGE (no cast needed: host sends int8) overlaps X0's SWDGE gen
            fetch(0, parts=((0, 4), (4, 8), (8, 16), (16, 19)))
            ylab8 = fixpool.tile([P, BPC * HW // P], i8)
            nc.sync.dma_start(
                out=ylab8[:].rearrange(
                    "p (b tp s) -> p b tp s", b=BPC, tp=TILES_PER_PLANE
                ),
                in_=yl[:].rearrange(
                    "b (tp p s) -> p b tp s", tp=TILES_PER_PLANE, p=P
                ),
            )
            fetch(1, parts=((0, 8), (8, 19)))

            ones = fixpool.tile([P, 1], fp16)
            nc.vector.memset(ones[:], 1.0)
            # per-partition bias constants for ACT D_Erf one-hot: -4c
            actbias = {}
            for c in sorted(act_cls):
                bt = fixpool.tile([P, 1], f32, name=f"nbias{c}")
                nc.vector.memset(bt[:], float(-4 * c))
                actbias[c] = bt

            psum_p = pspool.tile([P, S], f32, name="psump")  # full bank
            grams = [
                pspool.tile([P, S], f32, name=f"gram{g}") for g in range(5)
            ]

            def emit_ohy(t, OHY):
                yt = ylab8[:, t * S : (t + 1) * S]
                for c in range(C):
                    oc = OHY[:, c * S : (c + 1) * S]
                    if c in act_cls:
                        nc.scalar.activation(
                            out=oc, in_=yt, func=Act.Derivative_Erf,
                            bias=actbias[c][:], scale=4.0,
                        )
                    else:
                        nc.gpsimd.tensor_scalar(
                            out=oc, in0=yt, scalar1=float(c), scalar2=None,
                            op0=Alu.is_equal,
                        )

            def emit_ind(t, X, IND, mb):
                last = t == NTILES - 1
                if not last:
                    # sign-route tail classes off DVE: Pool subtract (f32)
                    # then ACT Sign -> {-1,0}; host un-biases the counts
                    for c in sorted(sgn_cls):
                        Dg = scpool.tile([P, S], f32, tag="dsub")
                        nc.gpsimd.tensor_tensor(
                            out=Dg[:], in0=X[:, c * S : (c + 1) * S],
                            in1=mb[:], op=Alu.subtract,
                        )
                        nc.scalar.activation(
                            out=IND[:, c * S : (c + 1) * S], in_=Dg[:],
                            func=Act.Sign,
                        )
                    groups = [(SGN, C)]
                else:
                    # final tile: everything on DVE, split by PSUM bank
                    # group so each bank's evac overlaps the next group
                    groups = [(0, 4), (4, 8), (8, 12), (12, 16), (16, C)]
                for lo, hi in groups:
                    n = hi - lo
                    mbap = mb[:]
                    mbview = bass.AP(
                        mbap.tensor, mbap.offset,
                        [mbap.ap[0], [0, n], mbap.ap[1]],
                    )
                    nc.vector.tensor_tensor(
                        out=IND[:, lo * S : hi * S].rearrange(
                            "p (c s) -> p c s", s=S
                        ),
                        in0=X[:, lo * S : hi * S].rearrange(
                            "p (c s) -> p c s", s=S
                        ),
                        in1=mbview,
                        op=Alu.is_ge,
                    )

            def fetch_ohy(t):
                O = ohpool.tile([P, C * S], fp16, tag="OHY")
                emit_ohy(t, O)
                ohtiles[t] = O

            fetch_ohy(0)
            for t in range(NTILES):
                if t + 2 < NTILES:
                    fetch(t + 2)
                X = xtiles.pop(t)
                IND = indpool.tile([P, C * S], fp16)

                # ---- max tree -> mb [P, S]; 8/8/3 split so the A part can
                # run on X[0:8S] alone (matches the split first-tile DMAs)
                t1 = treepool.tile([P, 9 * S], fp16, tag="t1")
                mb = smpool.tile([P, S], fp16, tag="mb")

                def vmax(out, a, b):
                    nc.vector.tensor_tensor(out=out, in0=a, in1=b, op=Alu.max)

                if t == 0:
                    # pair-sequential A half: usable after X[0:4S] even
                    vmax(t1[:, 0:2 * S], X[:, 0:2 * S], X[:, 2 * S:4 * S])
                    vmax(t1[:, 2 * S:4 * S], X[:, 4 * S:6 * S],
                         X[:, 6 * S:8 * S])
                else:
                    vmax(t1[:, 0:4 * S], X[:, 0:4 * S], X[:, 4 * S:8 * S])
                # L2A/L3A -> t1[0]
                vmax(t1[:, 0:2 * S], t1[:, 0:2 * S], t1[:, 2 * S:4 * S])
                vmax(t1[:, 0:S], t1[:, 0:S], t1[:, S:2 * S])
                # B half: classes 8..15 -> t1[4]
                vmax(t1[:, 4 * S:8 * S], X[:, 8 * S:12 * S],
                     X[:, 12 * S:16 * S])
                vmax(t1[:, 4 * S:6 * S], t1[:, 4 * S:6 * S],
                     t1[:, 6 * S:8 * S])
                vmax(t1[:, 4 * S:5 * S], t1[:, 4 * S:5 * S],
                     t1[:, 5 * S:6 * S])
                # C tail: classes 16..18
                vmax(t1[:, 8 * S:9 * S], X[:, 16 * S:17 * S],
                     X[:, 17 * S:18 * S])
                vmax(t1[:, 0:S], t1[:, 0:S], t1[:, 4 * S:5 * S])
                vmax(t1[:, 8 * S:9 * S], t1[:, 8 * S:9 * S],
                     X[:, 18 * S:19 * S])
                vmax(mb[:], t1[:, 0:S], t1[:, 8 * S:9 * S])

                emit_ind(t, X, IND, mb)

                if t + 1 < NTILES:
                    fetch_ohy(t + 1)
                OHY = ohtiles.pop(t)

                # ---- PE: count_p and gram accumulation.
                # start/stop once per PSUM bank: start zeroes the whole bank,
                # so only the first matmul touching a bank may set it.
                first = t == 0
                last = t == NTILES - 1
                for c in range(C):
                    gtile = grams[c // 4]
                    goff = (c % 4) * P
                    gfirst = c % 4 == 0
                    glast = c == C - 1 or c % 4 == 3
                    for a in range(NCHUNK):
                        lo = c * S + a * P
                        ind_ck = IND[:, lo : lo + P]
                        nc.tensor.matmul(
                            psum_p[:, c : c + 1], ind_ck, ones[:],
                            start=(first and c == 0 and a == 0),
                            stop=(last and c == C - 1 and a == NCHUNK - 1),
                            skip_group_check=True,
                        )
                        nc.tensor.matmul(
                            gtile[:, goff : goff + P],
                            OHY[:, lo : lo + P], ind_ck,
                            start=(first and gfirst and a == 0),
                            stop=(last and glast and a == NCHUNK - 1),
                            skip_group_check=True,
                        )

            # ---- psums out: per-bank copy + DMA as soon as each bank's last
            # matmul retires; all copies on ACT (idle during the drain)
            gr_sb = fixpool.tile([P, 5 * S], i16)
            for g in range(5):
                nc.scalar.activation(
                    out=gr_sb[:, g * S : (g + 1) * S], in_=grams[g][:],
                    func=Act.Copy,
                )
                nc.sync.dma_start(
                    out=out_gr[:, g * S : (g + 1) * S],
                    in_=gr_sb[:, g * S : (g + 1) * S],
                )
            cp_sb = fixpool.tile([P, C], f32)
            nc.scalar.activation(
                out=cp_sb[:], in_=psum_p[:, 0:C], func=Act.Copy
            )
            nc.sync.dma_start(out=out_cp[:], in_=cp_sb[:])
    return nc


def _split_excess_waits(nc, cap=1):
    """walrus codegen only fits `cap` inline sync-waits on most instruction
    structs; move the excess onto standalone EventSemaphore instructions
    executed just before, on the same engine queue."""
    import concourse.mybir as mybir

    n_split = 0
    for fn in nc.m.functions:
        for blk in fn.blocks:
            out = []
            for inst in blk.instructions:
                si = inst.sync_info
                if si is not None and len(si.on_wait) > cap:
                    waits = list(si.on_wait)
                    keep, excess = waits[-cap:], waits[:-cap]
                    for k, w in enumerate(excess):
                        es = mybir.InstEventSemaphore(
                            name=f"{inst.name}_wsplit{k}", ins=[], outs=[]
                        )
                        es.engine = inst.engine
                        es.sync_info = mybir.SyncInfo(on_wait=[w], on_update=[])
                        out.append(es)
                        n_split += 1
                    inst.sync_info = mybir.SyncInfo(
                        on_wait=keep, on_update=list(si.on_update)
                    )
                out.append(inst)
            blk.instructions[:] = out
    return n_split


def _get_nc():
    if "nc" not in _CACHE:
        nc = _build_nc()
        _split_excess_waits(nc)
        _CACHE["nc"] = nc
    return _CACHE["nc"]


def _run_device(y_pred, y, trace=False):
    from concourse.bass_utils import run_bass_kernel_spmd

    nc = _get_nc()
    xp = np.ascontiguousarray(y_pred.reshape(B, C, HW), dtype=np.float32)
    yi = np.ascontiguousarray(y.reshape(B, HW)).astype(np.int8)
    in_maps = []
    for i in range(NCORES):
        in_maps.append(
            {
                "y_pred": np.ascontiguousarray(xp[i * BPC : (i + 1) * BPC]),
                "y": np.ascontiguousarray(yi[i * BPC : (i + 1) * BPC]),
            }
        )
    return run_bass_kernel_spmd(
        nc, in_maps, core_ids=list(range(NCORES)), trace=trace
    )


def kernel(y_pred, y):
    res = _run_device(y_pred, y)
    count_p = np.zeros(C, dtype=np.float64)
    inter = np.zeros(C, dtype=np.float64)
    g0 = np.array([G0_ACT if c >= C - NACT else 1.0 for c in range(C)])
    yi = np.asarray(y).reshape(B, -1).astype(np.int64)
    # sign-route runs on tiles 0..NTILES-2: IND stored as IND-1 there
    npix_sgn = P * P * NCHUNK * (NTILES - 1)  # pixels in sign-route tiles
    tile7_lo = (TILES_PER_PLANE - 1) * P * S  # last tile = plane 1 tail
    for ci, r in enumerate(res.results):
        cp = r["cp_out"].astype(np.float64).sum(axis=0)
        gr = r["gram_out"].astype(np.float64)
        ycore = yi[ci * BPC : (ci + 1) * BPC]
        county_sgn = np.bincount(
            ycore[: BPC - 1].reshape(-1), minlength=C
        ).astype(np.float64) + np.bincount(
            ycore[BPC - 1, :tile7_lo], minlength=C
        ).astype(np.float64)
        for c in range(C):
            g = gr[:, (c // 4) * S + (c % 4) * P : (c // 4) * S + (c % 4) * P + P]
            tr = np.trace(g) / g0[c]
            if c < SGN:
                cp[c] += npix_sgn
                tr += county_sgn[c]
            inter[c] += tr
        count_p += cp
    count_y = np.bincount(yi.reshape(-1), minlength=C).astype(np.float64)

    count_p = count_p.astype(np.float32)
    count_y = count_y.astype(np.float32)
    inter = inter.astype(np.float32)
    union = count_y + count_p - inter
    eps = np.float32(EPS)
    dice = (np.float32(2.0) * inter + eps) / (union + eps)
    return np.float32(1.0) - np.mean(dice, dtype=np.float32)
